# revision 45
# baseline (speedup 1.0000x reference)
"""Trainium2 Bass kernel for the CACE message-passing GNN (nn_Cace_58291296141968).

Strategy (8 NeuronCores, SPMD), v2:
  - Receivers load-balanced onto 8 cores x 32 subtiles x 16 node slots; edges
    padded to CAP=192 slots/subtile (48 blocks of 128 slots per core).
  - sqrt(multinomial-prefactor) folded into the angular monomials so the
    symmetrizer is a plain sum of squares (no per-l prefactor multiply).
  - MP_NORM folded into the node table (A rows and V) so one UNSCALED bf16
    parity-split S_w per block serves both stage-1 and stage-2 seg-matmuls.
  - Stage-1: seg-sum + radial transform per subtile (bf16 matmuls), node A in
    f32; B0/chi computed per group of 8 subtiles; bf16 A table rows + V column
    repacked to DRAM and AllGathered in 4 row-slices overlapped with stage 1.
  - Stage-2: per 2 subtiles, dma_gather of 384 sender rows; msg_A via 8
    sigma-sliced matmuls per block, msg_B via chi-weighted angular rhs; A1
    accumulated on the gpsimd engine; B1 + output DMA per group of 8.
"""
import os
import numpy as np
from math import factorial, pi

import concourse.bacc as bacc
import concourse.bass as bass
import concourse.mybir as mybir
import concourse.tile as tile
from concourse.bass_utils import run_bass_kernel_spmd

# ---- problem constants (hardcoded; must match reference.py) ----
ZS = np.array([1, 6, 7, 8], dtype=np.int64)
NZ = 4
NAB = 3
CHAN = 9
MAX_L = 3
N_RBF = 8
N_RB = 8
CUTOFF = 5.5
MP_NORM = 1.0 / 10.0 ** 0.5
N_NODES = 4000
N_EDGES = 48000

def _make_l_list(max_l):
    lst = []
    for l in range(max_l + 1):
        for lx in range(l, -1, -1):
            for ly in range(l - lx, -1, -1):
                lst.append((lx, ly, l - lx - ly))
    return lst

L_LIST = _make_l_list(MAX_L)
N_L = len(L_LIST)                                   # 20
L_OF = np.array([sum(t) for t in L_LIST])
PREF = np.array([factorial(sum(t)) / (factorial(t[0]) * factorial(t[1]) * factorial(t[2]))
                 for t in L_LIST], dtype=np.float64)
L_RANGES = [(0, 1), (1, 4), (4, 10), (10, 20)]
# batched monomial chain: lists of (out_lo, out_hi, par_lo, comp)
_CHAIN_BATCH = [(4, 7, 1, 0), (7, 9, 2, 1), (9, 10, 3, 2),
                (10, 16, 4, 0), (16, 19, 7, 1), (19, 20, 9, 2)]

NC = 8
NSUB = 32
SUBN = 16
CAP = 192                # edge slots per subtile
ES = NSUB * CAP          # 6144 slots/core
EPB = 128
NBLK = ES // EPB         # 48 blocks/core
NROW = NSUB * SUBN       # 512 node rows/core
TABW = 1536              # table row: 1440 A + 9 V + pad (bytes % 256 == 0)
GB = 2                   # subtiles per gather call (3 blocks, 384 idx)
P = 128
F32 = mybir.dt.float32
BF16 = mybir.dt.bfloat16
I16 = mybir.dt.int16

_PROGRAM = None


def _block_ranges(s):
    """Blocks + partition ranges covering subtile s's 192 slots."""
    g2 = s // 2
    if s % 2 == 0:
        return [(3 * g2, 0, 128), (3 * g2 + 1, 0, 64)]
    return [(3 * g2 + 1, 64, 128), (3 * g2 + 2, 0, 128)]


# ================= host-side sharding prep (index work only) =================
def _prep(positions, shifts, atomic_numbers, edge_index):
    import heapq
    snd = np.asarray(edge_index[0]).astype(np.int64)
    rcv = np.asarray(edge_index[1]).astype(np.int64)
    an = np.asarray(atomic_numbers)
    species = np.searchsorted(ZS, an)
    indeg = np.bincount(rcv, minlength=N_NODES)
    order = np.argsort(-indeg, kind="stable")
    TS = NC * NSUB
    loads = np.zeros(TS, dtype=np.int64)
    counts = np.zeros(TS, dtype=np.int64)
    assign_sub = np.zeros(N_NODES, dtype=np.int64)
    assign_slot = np.zeros(N_NODES, dtype=np.int64)
    heap = [(0, t) for t in range(TS)]
    heapq.heapify(heap)
    for nd in order:
        pending = []
        while True:
            load, t = heapq.heappop(heap)
            if counts[t] < SUBN:
                break
            pending.append((load, t))
        assign_sub[nd] = t
        assign_slot[nd] = counts[t]
        counts[t] += 1
        loads[t] = load + indeg[nd]
        heapq.heappush(heap, (loads[t], t))
        for it in pending:
            heapq.heappush(heap, it)
    assert loads.max() <= CAP, f"subtile edge overflow: {loads.max()} > {CAP}"

    core_of = assign_sub // NSUB
    sub_of = assign_sub % NSUB
    node_row = core_of * NROW + sub_of * SUBN + assign_slot      # node -> global row
    node_of_row = np.full(NC * NROW, -1, dtype=np.int64)
    node_of_row[node_row] = np.arange(N_NODES)
    # table row in tabfull's (slice, core, 256-row) layout (contiguous AG slices)
    tab_row = (sub_of // 16) * (NC * 256) + core_of * 256 + (sub_of % 16) * SUBN + assign_slot

    e_sub = assign_sub[rcv]
    e_order = np.argsort(e_sub, kind="stable")
    bounds = np.searchsorted(e_sub[e_order], np.arange(TS + 1))

    pos = np.asarray(positions, dtype=np.float32)
    shf = np.asarray(shifts, dtype=np.float32)

    geo = np.zeros((NC, 9, ES), dtype=np.float32)                # [comp(SxyzRxyzShxyz), slot]
    geo[:, 3:6, :] = 1.0                                         # benign pad: R=(1,1,1), S=0
    recvoh = np.zeros((NC, SUBN, ES), dtype=np.float32)
    sendrow = np.zeros((NC, ES), dtype=np.int64)
    for t in range(TS):
        c = t // NSUB; s = t % NSUB
        es = e_order[bounds[t]:bounds[t + 1]]
        k = len(es)
        base = s * CAP
        geo[c, 0:3, base:base + k] = pos[snd[es]].T
        geo[c, 3:6, base:base + k] = pos[rcv[es]].T
        geo[c, 6:9, base:base + k] = shf[es].T
        recvoh[c, assign_slot[rcv[es]], base + np.arange(k)] = 1.0
        sendrow[c, base:base + k] = tab_row[snd[es]]

    # device edge-slot layout: slot -> (blk, p) with slot = blk*128 + p
    def to_pb(a):   # [NC, ..., ES] -> [NC, 128, ..., NBLK]
        a2 = a.reshape(a.shape[:-1] + (NBLK, EPB))               # [..., NBLK, 128]
        return np.moveaxis(a2, -1, 1)                            # [NC, 128, ..., NBLK]

    geo_in = np.ascontiguousarray(to_pb(geo).reshape(NC, P, 9 * NBLK))   # [NC, 128, (comp,blk)]
    # recv one-hot in (blk, n) layout: [NC, 128, NBLK, SUBN]
    recv_in = np.ascontiguousarray(
        to_pb(recvoh).transpose(0, 1, 3, 2).reshape(NC, P, NBLK * SUBN))
    # gather idx: per subtile 192 slots; idx k at partition k%16, col sub*12 + k//16
    gidx = np.zeros((NC, P, NSUB * 12), dtype=np.int16)
    for c in range(NC):
        w = sendrow[c].reshape(NSUB, 12, 16).astype(np.int16)    # [sub, k//16, k%16]
        packed = w.transpose(2, 0, 1).reshape(16, NSUB * 12)     # [k%16, (sub, k//16)]
        for g in range(8):
            gidx[c, g * 16:(g + 1) * 16, :] = packed
    # per-edge-slot sender species (pad -> 0) in device layout [NC, 128, NBLK]
    sendsp = np.zeros((NC, ES), dtype=np.int64)
    for t in range(TS):
        c = t // NSUB; s = t % NSUB
        es = e_order[bounds[t]:bounds[t + 1]]
        sendsp[c, s * CAP:s * CAP + len(es)] = species[snd[es]]
    sendsp_in = to_pb(sendsp)                                    # [NC, 128, NBLK]
    # per-node-row species (empty rows -> 0; all their uses are masked/zero)
    rowsp = np.zeros((NC, NROW), dtype=np.int64)
    msk = node_of_row >= 0
    rowsp.reshape(-1)[msk] = species[node_of_row[msk]]
    return dict(geo=geo_in, recv=recv_in, gidx=gidx, sendsp=sendsp_in, rowsp=rowsp,
                node_of_row=node_of_row, node_row=node_row)


def _consts():
    blkdiag = ((np.arange(P)[:, None] % 16) == (np.arange(P)[None, :] % 16)).astype(np.float32)
    nrow = np.tile((np.arange(1, N_RBF + 1) * pi / CUTOFF).astype(np.float32)[None, :], (P, 1))
    parc = np.zeros((P, 16), dtype=np.float32)                   # [par, r] keep r where r%2==par
    for par in range(2):
        for r in range(8):
            if r % 2 == par:
                parc[:, par * 8 + r] = 1.0
    sprow = np.tile(np.sqrt(PREF).astype(np.float32)[None, :], (P, 1))   # [128, 20]
    return np.concatenate([blkdiag, nrow, parc, sprow], axis=1)  # [128, 172]


# ================= device program =================
def _build(sim_mode=False):
    PH = int(os.environ.get("KPHASES", "63"))  # bit0 base,1 s1,2 node1,3 repack,4 s2,5 node2
    nc = bacc.Bacc("TRN2", target_bir_lowering=False, debug=False,
                   num_devices=(1 if sim_mode else NC))
    AF = mybir.ActivationFunctionType
    OP = mybir.AluOpType

    # x_main packs [geo 432 | recv 768 | cons 172 | wpack 308 | embse 144]
    NMAIN = 9 * NBLK + NBLK * SUBN + 172 + 308 + NBLK * NAB
    x_main = nc.dram_tensor("x_main", [P, NMAIN], F32, kind="ExternalInput")
    x_gidx = nc.dram_tensor("x_gidx", [P, NSUB * 12], I16, kind="ExternalInput")
    o_b0 = nc.dram_tensor("o_b0", [P, NSUB * 45], F32, kind="ExternalOutput")
    o_b1 = nc.dram_tensor("o_b1", [P, NSUB * 45], F32, kind="ExternalOutput")

    with tile.TileContext(nc) as tc:
        with (
            tc.tile_pool(name="persist", bufs=1) as pp,
            tc.tile_pool(name="work", bufs=2) as wp,
            tc.tile_pool(name="dram", bufs=1, space="DRAM") as dr,
        ):
            # ---------- loads (geo+recv first so the edge phase starts early) ----------
            main = pp.tile([P, NMAIN], F32)
            C1 = 9 * NBLK + NBLK * SUBN
            nc.sync.dma_start(main[:, 0:C1], x_main[:, 0:C1])
            nc.sync.dma_start(main[:, C1:NMAIN], x_main[:, C1:NMAIN])
            gidx = pp.tile([P, NSUB * 12], I16)
            nc.sync.dma_start(gidx[:], x_gidx[:])
            o = 0
            geo = main[:, o:o + 9 * NBLK]; o += 9 * NBLK
            recvs = main[:, o:o + NBLK * SUBN]; o += NBLK * SUBN
            cons = main[:, o:o + 172]; o += 172
            wpack = main[:, o:o + 308]; o += 308
            embsE = main[:, o:o + NBLK * NAB]; o += NBLK * NAB
            blkdiag = cons[:, 0:128]
            nrow = cons[:, 128:136]
            parc = cons[:, 136:152]
            sprow = cons[:, 152:172]
            EM = wpack[:, 212:308]

            # ---------- one-time derived weights ----------
            rtl16 = []
            for l in range(MAX_L + 1):
                rtl_t = pp.tile([P, P], BF16, tag=f"rtl{l}")
                rtl16.append(rtl_t)
                nc.vector.tensor_tensor(
                    out=rtl_t[:].rearrange("p (s n) -> p s n", s=8),
                    in0=wpack[:, l * 8:(l + 1) * 8][:, :, None].to_broadcast([P, 8, 16]),
                    in1=blkdiag.rearrange("p (s n) -> p s n", s=8),
                    op=OP.mult)
            WT16 = pp.tile([P, 180], BF16)
            nc.scalar.copy(WT16[:], wpack[:, 32:212])

            # ---------- per-edge base phase ----------
            D = pp.tile([P, 3 * NBLK], F32)
            nc.vector.tensor_tensor(out=D[:], in0=geo[:, 3 * NBLK:6 * NBLK],
                                    in1=geo[:, 0:3 * NBLK], op=OP.subtract)
            nc.vector.tensor_tensor(out=D[:], in0=D[:], in1=geo[:, 6 * NBLK:9 * NBLK], op=OP.add)
            sq = wp.tile([P, 3 * NBLK], F32, tag="sq")
            nc.vector.tensor_tensor(out=sq[:], in0=D[:], in1=D[:], op=OP.mult)
            r2 = wp.tile([P, NBLK], F32, tag="r2")
            nc.vector.tensor_tensor(out=r2[:], in0=sq[:, 0:NBLK], in1=sq[:, NBLK:2 * NBLK], op=OP.add)
            nc.vector.tensor_tensor(out=r2[:], in0=r2[:], in1=sq[:, 2 * NBLK:3 * NBLK], op=OP.add)
            rr = wp.tile([P, NBLK], F32, tag="rr")
            nc.scalar.activation(rr[:], r2[:], AF.Sqrt)
            rinv = pp.tile([P, NBLK], F32)
            nc.vector.reciprocal(rinv[:], rr[:])
            uu = wp.tile([P, NBLK], F32, tag="uu")
            nc.vector.tensor_scalar_mul(uu[:], rr[:], 1.0 / CUTOFF)
            U = pp.tile([P, 3 * NBLK], F32)
            nc.vector.tensor_tensor(
                out=U[:].rearrange("p (c b) -> p c b", c=3),
                in0=D[:].rearrange("p (c b) -> p c b", c=3),
                in1=rinv[:, None, :].to_broadcast([P, 3, NBLK]), op=OP.mult)
            # angular monomials ang [128, (blk, i)] scaled by sqrt(PREF) and the
            # G1 edge features: run on the (otherwise idle) gpsimd engine in
            # parallel with the DVE bessel chain, so the first PE matmuls and
            # the first table repack start as early as possible
            ang = pp.tile([P, NBLK * N_L], F32)
            angv = ang[:].rearrange("p (b i) -> p b i", i=N_L)
            nc.gpsimd.tensor_scalar(out=angv[:, :, 0], in0=uu[:], scalar1=0.0, scalar2=1.0,
                                    op0=OP.mult, op1=OP.add)
            nc.gpsimd.tensor_copy(
                angv[:, :, 1:4],
                U[:].rearrange("p (c b) -> p b c", c=3))
            for lo, hi, plo, c in _CHAIN_BATCH:
                cnt = hi - lo
                nc.gpsimd.tensor_tensor(
                    out=angv[:, :, lo:hi],
                    in0=angv[:, :, plo:plo + cnt],
                    in1=U[:, c * NBLK:(c + 1) * NBLK][:, :, None].to_broadcast([P, NBLK, cnt]),
                    op=OP.mult)
            nc.gpsimd.tensor_tensor(
                out=angv[:],
                in0=angv[:],
                in1=sprow[:, None, :].to_broadcast([P, NBLK, N_L]),
                op=OP.mult)
            G1 = pp.tile([P, NBLK * N_L * NAB], BF16)

            def build_g1():
                # G1 [128, (blk, i, a)] bf16 (DVE; emitted after sw2 group 0)
                nc.vector.tensor_tensor(
                    out=G1[:].rearrange("p (b i a) -> p b i a", i=N_L, a=NAB),
                    in0=angv[:, :, :, None].to_broadcast([P, NBLK, N_L, NAB]),
                    in1=embsE[:].rearrange("p (b a) -> p b a", a=NAB)[:, :, None, :].to_broadcast([P, NBLK, N_L, NAB]),
                    op=OP.mult)
            # bessel args [128, (blk, r)] + range reduction to [-pi, pi)
            arg = wp.tile([P, NBLK * 8], F32, tag="arg")
            nc.vector.tensor_tensor(
                out=arg[:].rearrange("p (b r) -> p b r", r=8),
                in0=rr[:, :, None].to_broadcast([P, NBLK, 8]),
                in1=nrow[:, None, :].to_broadcast([P, NBLK, 8]), op=OP.mult)
            ge = wp.tile([P, NBLK * 8], F32, tag="ge")
            for thr, sub in ((4 * pi, 4 * pi), (2 * pi, 2 * pi), (pi, 2 * pi)):
                nc.vector.tensor_scalar(out=ge[:], in0=arg[:], scalar1=float(thr),
                                        scalar2=float(sub), op0=OP.is_ge, op1=OP.mult)
                nc.vector.tensor_tensor(out=arg[:], in0=arg[:], in1=ge[:], op=OP.subtract)
            sinv = wp.tile([P, NBLK * 8], F32, tag="sinv")
            nc.scalar.activation(sinv[:], arg[:], AF.Sin)
            # cutoff polynomial
            u2 = wp.tile([P, NBLK], F32, tag="u2")
            nc.vector.tensor_tensor(out=u2[:], in0=uu[:], in1=uu[:], op=OP.mult)
            a1 = wp.tile([P, NBLK], F32, tag="a1")
            nc.vector.tensor_scalar(out=a1[:], in0=uu[:], scalar1=-48.0, scalar2=28.0,
                                    op0=OP.mult, op1=OP.add)
            g21 = wp.tile([P, NBLK], F32, tag="g21")
            nc.vector.tensor_scalar_mul(g21[:], u2[:], 21.0)
            nc.vector.tensor_tensor(out=g21[:], in0=g21[:], in1=a1[:], op=OP.add)
            u6 = wp.tile([P, NBLK], F32, tag="u6")
            nc.vector.tensor_tensor(out=u6[:], in0=u2[:], in1=u2[:], op=OP.mult)
            nc.vector.tensor_tensor(out=u6[:], in0=u6[:], in1=u2[:], op=OP.mult)
            fc = wp.tile([P, NBLK], F32, tag="fc")
            nc.vector.tensor_tensor(out=fc[:], in0=u6[:], in1=g21[:], op=OP.mult)
            nc.vector.tensor_scalar(out=fc[:], in0=fc[:], scalar1=-1.0, scalar2=1.0,
                                    op0=OP.mult, op1=OP.add)
            lt = wp.tile([P, NBLK], F32, tag="lt")
            nc.vector.tensor_scalar(out=lt[:], in0=uu[:], scalar1=1.0, scalar2=None, op0=OP.is_lt)
            nc.vector.tensor_tensor(out=fc[:], in0=fc[:], in1=lt[:], op=OP.mult)
            scal = wp.tile([P, NBLK], F32, tag="scal")
            nc.vector.tensor_tensor(out=scal[:], in0=rinv[:], in1=fc[:], op=OP.mult)
            nc.vector.tensor_scalar_mul(scal[:], scal[:], float(np.sqrt(2.0 / CUTOFF)))
            rc = pp.tile([P, NBLK * 8], F32)
            nc.vector.tensor_tensor(
                out=rc[:].rearrange("p (b r) -> p b r", r=8),
                in0=sinv[:].rearrange("p (b r) -> p b r", r=8),
                in1=scal[:, :, None].to_broadcast([P, NBLK, 8]), op=OP.mult)
            # parity-masked rc: rcEO [128, (blk, par, r)]
            rcEO = pp.tile([P, NBLK * 16], F32)
            nc.vector.tensor_tensor(
                out=rcEO[:].rearrange("p (b q r) -> p b q r", q=2, r=8),
                in0=rc[:].rearrange("p (b r) -> p b r", r=8)[:, :, None, :].to_broadcast([P, NBLK, 2, 8]),
                in1=parc.rearrange("p (q r) -> p q r", q=2)[:, None, :, :].to_broadcast([P, NBLK, 2, 8]),
                op=OP.mult)
            # unified parity-split S_w, bf16, persistent (used by both stages);
            # built per 6-block group inside the stage-1 loop, alternating
            # DVE/gpsimd to balance engine load
            sw2 = pp.tile([P, NBLK * 256], BF16)

            def build_sw2(g4):
                bs = slice(g4 * 6, g4 * 6 + 6)
                eng = nc.vector
                eng.tensor_tensor(
                    out=sw2[:, g4 * 1536:(g4 + 1) * 1536].rearrange(
                        "p (b qr n) -> p b qr n", qr=16, n=16),
                    in0=recvs[:].rearrange("p (b n) -> p b n", n=SUBN)[:, bs, None, :].to_broadcast([P, 6, 16, 16]),
                    in1=rcEO[:].rearrange("p (b qr) -> p b qr", qr=16)[:, bs, :, None].to_broadcast([P, 6, 16, 16]),
                    op=OP.mult)

            def symmetrize_pool(bv, av, sv, ns):
                # bv [P,ns,5,c]; av/sv [P,ns,20,c]: sum-of-squares tree on gpsimd
                s5 = wp.tile([P, 8 * 5 * CHAN], F32, tag="ps5")
                v5 = s5[:].rearrange("p (s i c) -> p s i c", i=5, c=CHAN)[:, 0:ns]
                s3 = wp.tile([P, 8 * 3 * CHAN], F32, tag="ps3")
                v3 = s3[:].rearrange("p (s i c) -> p s i c", i=3, c=CHAN)[:, 0:ns]
                TT = nc.gpsimd.tensor_tensor
                nc.vector.tensor_copy(bv[:, :, 0, :], av[:, :, 0, :])
                nc.gpsimd.tensor_copy(bv[:, :, 1, :], sv[:, :, 0, :])
                # l=1: i 1..4
                TT(out=v3[:, :, 0, :], in0=sv[:, :, 1, :], in1=sv[:, :, 2, :], op=OP.add)
                TT(out=bv[:, :, 2, :], in0=v3[:, :, 0, :], in1=sv[:, :, 3, :], op=OP.add)
                # l=2: i 4..10
                TT(out=v3[:], in0=sv[:, :, 4:7, :], in1=sv[:, :, 7:10, :], op=OP.add)
                TT(out=v5[:, :, 0, :], in0=v3[:, :, 0, :], in1=v3[:, :, 1, :], op=OP.add)
                TT(out=bv[:, :, 3, :], in0=v5[:, :, 0, :], in1=v3[:, :, 2, :], op=OP.add)
                # l=3: i 10..20
                TT(out=v5[:], in0=sv[:, :, 10:15, :], in1=sv[:, :, 15:20, :], op=OP.add)
                TT(out=v3[:, :, 0:2, :], in0=v5[:, :, 0:2, :], in1=v5[:, :, 2:4, :], op=OP.add)
                TT(out=v3[:, :, 2, :], in0=v3[:, :, 0, :], in1=v3[:, :, 1, :], op=OP.add)
                TT(out=bv[:, :, 4, :], in0=v3[:, :, 2, :], in1=v5[:, :, 4, :], op=OP.add)

            def symmetrize_dve(bv, av, sv):
                nc.vector.tensor_copy(bv[:, :, 0, :], av[:, :, 0, :])
                for l, (a, b) in enumerate(L_RANGES):
                    nc.vector.tensor_reduce(
                        out=bv[:, :, l + 1, :],
                        in_=sv[:, :, a:b, :].transpose([0, 1, 3, 2]),
                        axis=mybir.AxisListType.X, op=OP.add)

            A_all = pp.tile([P, NSUB * 180], F32)
            A16 = pp.tile([P, NSUB * 180], BF16)
            mem16 = pp.tile([P, NSUB * 180], BF16)
            B0_all = pp.tile([P, NSUB * 45], F32)
            B1_all = pp.tile([P, NSUB * 45], F32)
            red1 = pp.tile([P, NSUB * CHAN], F32)
            chic = pp.tile([16, NSUB * CHAN], F32)
            Vsb = pp.tile([16, NSUB * CHAN], BF16)

            tabsh = dr.tile([NROW, TABW], BF16)
            tabfull = dr.tile([NC * NROW, TABW], BF16)

            # ---------- stage 1: seg-sum + RT + A; node-level per 8 subtiles ----------
            s1ctx = tc.tile_pool(name="ps_s1", bufs=2, space="PSUM")
            ps_s1 = s1ctx.__enter__()
            if PH & 2:
                build_sw2(0)
                build_g1()
                build_sw2(1)
            for g8 in range(4 if (PH & 2) else 0):
                if g8 < 3:
                    # prefetch next group's S_w on DVE before this group's
                    # node-level ops (which wait on PE/Act) enter the queue
                    build_sw2(2 * g8 + 2)
                    build_sw2(2 * g8 + 3)
                t0g = ps_s1.tile([P, 480], F32, space="PSUM", tag="t0g")
                for j in range(8):
                    s = g8 * 8 + j
                    osl = slice(j * 60, (j + 1) * 60)
                    mms = [(blk, p0, p1, q) for (blk, p0, p1) in _block_ranges(s) for q in (0, 1)]
                    for mi, (blk, p0, p1, q) in enumerate(mms):
                        nc.tensor.matmul(t0g[:, osl], lhsT=sw2[p0:p1, blk * 256 + q * 128: blk * 256 + q * 128 + 128],
                                         rhs=G1[p0:p1, blk * 60:(blk + 1) * 60],
                                         start=(mi == 0), stop=(mi == len(mms) - 1))
                t0c = wp.tile([P, 480], BF16, tag="t0c")
                nc.scalar.copy(t0c[:], t0g[:])
                t1g = ps_s1.tile([P, 480], F32, space="PSUM", tag="t1g")
                t0cv = t0c[:].rearrange("p (j f) -> p j f", f=60)
                t1gv = t1g[:].rearrange("p (j f) -> p j f", f=60)
                for l, (a, b) in enumerate(L_RANGES):
                    nc.tensor.matmul(t1gv[:, :, a * NAB:b * NAB],
                                     lhsT=rtl16[l][:], rhs=t0cv[:, :, a * NAB:b * NAB],
                                     start=True, stop=True)
                if not (PH & 4):
                    continue
                # ---- group node-level: A, A16, B0, chi, V, repack, AG slice ----
                sl = slice(g8 * 1440, (g8 + 1) * 1440)
                sl45 = slice(g8 * 360, (g8 + 1) * 360)
                sl9 = slice(g8 * 72, (g8 + 1) * 72)
                nc.vector.tensor_tensor(
                    out=A_all[:, sl].rearrange("p (j ia b) -> p j ia b", j=8, b=NAB),
                    in0=t1g[:].rearrange("p (j ia) -> p j ia", j=8)[:, :, :, None].to_broadcast([P, 8, 60, NAB]),
                    in1=EM[:, g8 * 24:(g8 + 1) * 24].rearrange("p (j b) -> p j b", b=NAB)[:, :, None, :].to_broadcast([P, 8, 60, NAB]),
                    op=OP.mult)
                nc.scalar.mul(A16[:, sl], A_all[:, sl], float(MP_NORM))
                scr = wp.tile([P, 1440], F32, tag="scr")
                nc.scalar.activation(scr[:], A_all[:, sl], AF.Square)
                bv = B0_all[:, sl45].rearrange("p (s l c) -> p s l c", l=5, c=CHAN)
                av = A_all[:, sl].rearrange("p (s i c) -> p s i c", i=N_L, c=CHAN)
                sv = scr[:].rearrange("p (s i c) -> p s i c", i=N_L, c=CHAN)
                if g8 == 3:
                    symmetrize_dve(bv, av, sv)   # shortest path into the last AG slice
                else:
                    symmetrize_pool(bv, av, sv, 8)
                nc.vector.tensor_reduce(
                    out=red1[:, sl9].rearrange("p (s c) -> p s c", c=CHAN),
                    in_=bv.transpose([0, 1, 3, 2]),
                    axis=mybir.AxisListType.X, op=OP.add)
                chips = ps_s1.tile([16, 72], F32, space="PSUM", tag="chips")
                nc.tensor.matmul(chips[:], lhsT=blkdiag[:, 0:16], rhs=red1[:, sl9],
                                 start=True, stop=True)
                nc.scalar.mul(chic[:, sl9], chips[:], float(MP_NORM))
                nc.vector.tensor_tensor(
                    out=Vsb[:, sl9].rearrange("p (s a b) -> p s a b", a=NAB, b=NAB),
                    in0=chic[:, sl9].rearrange("p (s a b) -> p s a b", a=NAB, b=NAB),
                    in1=EM[0:16, g8 * 24:(g8 + 1) * 24].rearrange("p (s a) -> p s a", a=NAB)[:, :, :, None].to_broadcast([16, 8, NAB, NAB]),
                    op=OP.mult)
                if (PH & 8) and g8 % 2 == 1:
                    # repack super-group: A rows + V column for 16 subtiles
                    # (256 table rows), 8+1 DMAs; then one AllGather slice
                    sg = g8 // 2
                    ssl = slice(sg * 2880, (sg + 1) * 2880)
                    ssl9 = slice(sg * 144, (sg + 1) * 144)
                    for sp in range(8):
                        nc.sync.dma_start(
                            out=tabsh[:].rearrange("(s n) w -> n s w", n=SUBN)[:, sg * 16:(sg + 1) * 16, sp * 180:(sp + 1) * 180],
                            in_=A16[sp * 16:(sp + 1) * 16, ssl].rearrange("n (s f) -> n s f", f=180))
                    nc.sync.dma_start(
                        out=tabsh[:].rearrange("(s n) w -> n s w", n=SUBN)[:, sg * 16:(sg + 1) * 16, 1440:1449],
                        in_=Vsb[:, ssl9].rearrange("n (s c) -> n s c", c=CHAN))
                    rsl = slice(sg * 256, (sg + 1) * 256)
                    if sim_mode:
                        # stand-in for the sliced AllGather: one DMA per slice
                        # re-reading the shard 4x models the measured ~17us
                        # 8-core AG of the 1.5MB/rank shard (same total bytes)
                        nc.sync.dma_start(
                            tabfull[sg * NC * 256: sg * NC * 256 + 1024, :].rearrange(
                                "(c r) w -> c r w", c=4),
                            tabsh[rsl, :][None, :, :].to_broadcast([4, 256, TABW]))
                    else:
                        # tabfull rows are (slice, core, 256): slice output is
                        # the contiguous rank-major block for this slice
                        nc.gpsimd.collective_compute(
                            "AllGather", mybir.AluOpType.bypass,
                            replica_groups=[list(range(NC))],
                            ins=[tabsh[rsl, :]],
                            outs=[tabfull[sg * NC * 256:(sg + 1) * NC * 256, :]])
            nc.sync.dma_start(o_b0[:], B0_all[:])
            # memory term (bf16 fast path; WT pre-divided by MP_NORM on host)
            nc.vector.tensor_tensor(
                out=mem16[:].rearrange("p (s f) -> p s f", f=180),
                in0=A16[:].rearrange("p (s f) -> p s f", f=180),
                in1=WT16[:, None, :].to_broadcast([P, NSUB, 180]),
                op=OP.mult)
            s1ctx.__exit__(None, None, None)

            # ---------- stage 2 ----------
            s2ctx = tc.tile_pool(name="ps_s2", bufs=2, space="PSUM")
            ps_s2 = s2ctx.__enter__()
            A1f = None
            for gg in range(NSUB // GB if (PH & 16) else 0):
                g8 = gg // 4
                if gg % 2 == 0:
                    A1f = wp.tile([P, 720], F32, tag="a1f", bufs=3)
                gat = wp.tile([P, 3, TABW], BF16, tag="gat", bufs=4)
                nc.gpsimd.dma_gather(gat[:], tabfull[:],
                                     gidx[:, gg * 24:(gg + 1) * 24],
                                     GB * CAP, GB * CAP, TABW)
                G2 = wp.tile([P, 3, 180], BF16, tag="g2", bufs=3)
                nc.vector.tensor_tensor(
                    out=G2[:].rearrange("p b (i c) -> p b i c", c=CHAN),
                    in0=angv[:, 3 * gg:3 * gg + 3, :][:, :, :, None].to_broadcast([P, 3, N_L, CHAN]),
                    in1=gat[:, :, 1440:1449][:, :, None, :].to_broadcast([P, 3, N_L, CHAN]),
                    op=OP.mult)
                t2pair = ps_s2.tile([P, 360], F32, space="PSUM", tag="t2", bufs=3)
                a1pair = ps_s2.tile([P, 360], F32, space="PSUM", tag="a1p", bufs=3)
                t2s = wp.tile([P, 360], BF16, tag="t2s", bufs=3)
                for s2 in range(GB):
                    s = gg * GB + s2
                    osl = slice(s2 * 180, (s2 + 1) * 180)
                    ranges = _block_ranges(s)
                    mms = [(blk, p0, p1, q) for (blk, p0, p1) in ranges for q in (0, 1)]
                    for mi, (blk, p0, p1, q) in enumerate(mms):
                        nc.tensor.matmul(
                            t2pair[:, osl],
                            lhsT=sw2[p0:p1, blk * 256 + q * 128: blk * 256 + q * 128 + 128],
                            rhs=G2[p0:p1, blk - 3 * gg, :],
                            start=(mi == 0), stop=(mi == len(mms) - 1))
                    for ri, (blk, p0, p1) in enumerate(ranges):
                        bloc = blk - 3 * gg
                        for sig in (0, 2, 4, 6, 1, 3, 5, 7):
                            k, par = sig // 2, sig % 2
                            nc.tensor.matmul(
                                a1pair[k * 32:(k + 1) * 32, osl],
                                lhsT=sw2[p0:p1, blk * 256 + par * 128 + k * 32: blk * 256 + par * 128 + (k + 1) * 32],
                                rhs=gat[p0:p1, bloc, sig * 180:(sig + 1) * 180],
                                start=(ri == 0 and par == 0), stop=False,
                                tile_position=(p0, k * 32))
                    nc.vector.tensor_tensor(
                        out=t2s[:, osl].rearrange("p (i a b) -> p i a b", a=NAB, b=NAB),
                        in0=t2pair[:, osl].rearrange("p (i a b) -> p i a b", a=NAB, b=NAB),
                        in1=EM[:, s * NAB:(s + 1) * NAB][:, None, None, :].to_broadcast([P, N_L, NAB, NAB]),
                        op=OP.mult)
                t2sv = t2s[:].rearrange("p (s2 f) -> p s2 f", f=180)
                a1v = a1pair[:].rearrange("p (s2 f) -> p s2 f", f=180)
                for l, (a, b) in enumerate(L_RANGES):
                    nc.tensor.matmul(a1v[:, :, a * CHAN:b * CHAN],
                                     lhsT=rtl16[l][:], rhs=t2sv[:, :, a * CHAN:b * CHAN],
                                     start=False, stop=(l == MAX_L))
                nc.vector.tensor_tensor(
                    out=A1f[:, (gg % 2) * 360:(gg % 2 + 1) * 360],
                    in0=a1pair[:], in1=mem16[:, gg * 360:(gg + 1) * 360], op=OP.add)
                if gg % 2 < 1 or not (PH & 32):
                    continue
                # ---- stage 2 node-level per 4 subtiles: B1 + output ----
                h4 = gg // 2
                sl45 = slice(h4 * 180, (h4 + 1) * 180)
                scr = wp.tile([P, 720], F32, tag="scr1", bufs=3)
                nc.scalar.activation(scr[:], A1f[:], AF.Square)
                bv = B1_all[:, sl45].rearrange("p (s l c) -> p s l c", l=5, c=CHAN)
                av = A1f[:].rearrange("p (s i c) -> p s i c", i=N_L, c=CHAN)
                sv = scr[:].rearrange("p (s i c) -> p s i c", i=N_L, c=CHAN)
                symmetrize_dve(bv, av, sv)
                nc.sync.dma_start(o_b1[:, sl45], B1_all[:, sl45])
            s2ctx.__exit__(None, None, None)
            if not (PH & 4):
                nc.sync.dma_start(o_b0[:, 0:172], cons[:])
            if not (PH & 32):
                nc.sync.dma_start(o_b1[:, 0:172], cons[:])
    nc.compile()
    return nc


# ================= public entry =================
def kernel(positions, shifts, W_emb, W_rt, W_nm, atomic_numbers, edge_index):
    global _PROGRAM
    prep = _prep(positions, shifts, atomic_numbers, edge_index)
    consts = _consts()
    if _PROGRAM is None:
        _PROGRAM = _build()
    nc = _PROGRAM
    wemb = np.asarray(W_emb, dtype=np.float32)
    wrt = np.asarray(W_rt, dtype=np.float32)
    wnm = np.asarray(W_nm, dtype=np.float32)
    # host-replicated weight patterns (pure tiling/gathers of the small weights)
    pg = np.arange(P) // 16                                   # r|s' group per partition
    rtlw = wrt[:, pg, :].transpose(1, 0, 2).reshape(P, 32)    # [p, (l, s')] = W_rt[l, p//16, s']
    wtp = wnm[0, pg][:, L_OF, :].reshape(P, 180) / np.float32(MP_NORM)
    in_maps = []
    for c in range(NC):
        em = wemb[prep["rowsp"][c].reshape(NSUB, SUBN)]       # [sub, n, a]
        em = em[:, np.arange(P) % 16, :].transpose(1, 0, 2).reshape(P, NSUB * NAB)
        wpack = np.concatenate([rtlw, wtp, em], axis=1).astype(np.float32)
        embse = wemb[prep["sendsp"][c]].reshape(P, NBLK * NAB).astype(np.float32)
        main = np.ascontiguousarray(np.concatenate(
            [prep["geo"][c], prep["recv"][c], consts, wpack, embse],
            axis=1).astype(np.float32))
        in_maps.append(dict(x_main=main, x_gidx=prep["gidx"][c]))
    res = run_bass_kernel_spmd(nc, in_maps, list(range(NC))).results
    # unshard: [128=(s',n), (sub, l, c)] -> node rows
    out = np.zeros((N_NODES, N_RB, 5, CHAN, 2), dtype=np.float32)
    node_of_row = prep["node_of_row"]
    for c in range(NC):
        for mp, name in ((0, "o_b0"), (1, "o_b1")):
            arr = res[c][name].reshape(8, SUBN, NSUB, 5, CHAN)    # [s', n, sub, l, ch]
            rows = arr.transpose(2, 1, 0, 3, 4).reshape(NROW, N_RB, 5, CHAN)
            valid = node_of_row[c * NROW:(c + 1) * NROW] >= 0
            out[node_of_row[c * NROW:(c + 1) * NROW][valid], :, :, :, mp] = rows[valid]
    return out


# revision 48
# speedup vs baseline: 1.0115x; 1.0115x over previous
"""Trainium2 Bass kernel for the CACE message-passing GNN (nn_Cace_58291296141968).

Strategy (8 NeuronCores, SPMD), v2:
  - Receivers load-balanced onto 8 cores x 32 subtiles x 16 node slots; edges
    padded to CAP=192 slots/subtile (48 blocks of 128 slots per core).
  - sqrt(multinomial-prefactor) folded into the angular monomials so the
    symmetrizer is a plain sum of squares (no per-l prefactor multiply).
  - MP_NORM folded into the node table (A rows and V) so one UNSCALED bf16
    parity-split S_w per block serves both stage-1 and stage-2 seg-matmuls.
  - Stage-1: seg-sum + radial transform per subtile (bf16 matmuls), node A in
    f32; B0/chi computed per group of 8 subtiles; bf16 A table rows + V column
    repacked to DRAM and AllGathered in 4 row-slices overlapped with stage 1.
  - Stage-2: per 2 subtiles, dma_gather of 384 sender rows; msg_A via 8
    sigma-sliced matmuls per block, msg_B via chi-weighted angular rhs; A1
    accumulated on the gpsimd engine; B1 + output DMA per group of 8.
"""
import os
import numpy as np
from math import factorial, pi

import concourse.bacc as bacc
import concourse.bass as bass
import concourse.mybir as mybir
import concourse.tile as tile
from concourse.bass_utils import run_bass_kernel_spmd

# ---- problem constants (hardcoded; must match reference.py) ----
ZS = np.array([1, 6, 7, 8], dtype=np.int64)
NZ = 4
NAB = 3
CHAN = 9
MAX_L = 3
N_RBF = 8
N_RB = 8
CUTOFF = 5.5
MP_NORM = 1.0 / 10.0 ** 0.5
N_NODES = 4000
N_EDGES = 48000

def _make_l_list(max_l):
    lst = []
    for l in range(max_l + 1):
        for lx in range(l, -1, -1):
            for ly in range(l - lx, -1, -1):
                lst.append((lx, ly, l - lx - ly))
    return lst

L_LIST = _make_l_list(MAX_L)
N_L = len(L_LIST)                                   # 20
L_OF = np.array([sum(t) for t in L_LIST])
PREF = np.array([factorial(sum(t)) / (factorial(t[0]) * factorial(t[1]) * factorial(t[2]))
                 for t in L_LIST], dtype=np.float64)
L_RANGES = [(0, 1), (1, 4), (4, 10), (10, 20)]
# batched monomial chain: lists of (out_lo, out_hi, par_lo, comp)
_CHAIN_BATCH = [(4, 7, 1, 0), (7, 9, 2, 1), (9, 10, 3, 2),
                (10, 16, 4, 0), (16, 19, 7, 1), (19, 20, 9, 2)]

NC = 8
NSUB = 32
SUBN = 16
CAP = 192                # edge slots per subtile
ES = NSUB * CAP          # 6144 slots/core
EPB = 128
NBLK = ES // EPB         # 48 blocks/core
NROW = NSUB * SUBN       # 512 node rows/core
TABW = 1536              # table row: 1440 A + 9 V + pad (bytes % 256 == 0)
GB = 2                   # subtiles per gather call (3 blocks, 384 idx)
P = 128
F32 = mybir.dt.float32
BF16 = mybir.dt.bfloat16
I16 = mybir.dt.int16

_PROGRAM = None


def _block_ranges(s):
    """Blocks + partition ranges covering subtile s's 192 slots."""
    g2 = s // 2
    if s % 2 == 0:
        return [(3 * g2, 0, 128), (3 * g2 + 1, 0, 64)]
    return [(3 * g2 + 1, 64, 128), (3 * g2 + 2, 0, 128)]


# ================= host-side sharding prep (index work only) =================
def _prep(positions, shifts, atomic_numbers, edge_index):
    import heapq
    snd = np.asarray(edge_index[0]).astype(np.int64)
    rcv = np.asarray(edge_index[1]).astype(np.int64)
    an = np.asarray(atomic_numbers)
    species = np.searchsorted(ZS, an)
    indeg = np.bincount(rcv, minlength=N_NODES)
    order = np.argsort(-indeg, kind="stable")
    TS = NC * NSUB
    loads = np.zeros(TS, dtype=np.int64)
    counts = np.zeros(TS, dtype=np.int64)
    assign_sub = np.zeros(N_NODES, dtype=np.int64)
    assign_slot = np.zeros(N_NODES, dtype=np.int64)
    heap = [(0, t) for t in range(TS)]
    heapq.heapify(heap)
    for nd in order:
        pending = []
        while True:
            load, t = heapq.heappop(heap)
            if counts[t] < SUBN:
                break
            pending.append((load, t))
        assign_sub[nd] = t
        assign_slot[nd] = counts[t]
        counts[t] += 1
        loads[t] = load + indeg[nd]
        heapq.heappush(heap, (loads[t], t))
        for it in pending:
            heapq.heappush(heap, it)
    assert loads.max() <= CAP, f"subtile edge overflow: {loads.max()} > {CAP}"

    core_of = assign_sub // NSUB
    sub_of = assign_sub % NSUB
    node_row = core_of * NROW + sub_of * SUBN + assign_slot      # node -> global row
    node_of_row = np.full(NC * NROW, -1, dtype=np.int64)
    node_of_row[node_row] = np.arange(N_NODES)
    # table row in tabfull's (slice, core, 256-row) layout (contiguous AG slices)
    tab_row = (sub_of // 16) * (NC * 256) + core_of * 256 + (sub_of % 16) * SUBN + assign_slot

    e_sub = assign_sub[rcv]
    e_order = np.argsort(e_sub, kind="stable")
    bounds = np.searchsorted(e_sub[e_order], np.arange(TS + 1))

    pos = np.asarray(positions, dtype=np.float32)
    shf = np.asarray(shifts, dtype=np.float32)

    geo = np.zeros((NC, 9, ES), dtype=np.float32)                # [comp(SxyzRxyzShxyz), slot]
    geo[:, 3:6, :] = 1.0                                         # benign pad: R=(1,1,1), S=0
    recvoh = np.zeros((NC, SUBN, ES), dtype=np.float32)
    sendrow = np.zeros((NC, ES), dtype=np.int64)
    for t in range(TS):
        c = t // NSUB; s = t % NSUB
        es = e_order[bounds[t]:bounds[t + 1]]
        k = len(es)
        base = s * CAP
        geo[c, 0:3, base:base + k] = pos[snd[es]].T
        geo[c, 3:6, base:base + k] = pos[rcv[es]].T
        geo[c, 6:9, base:base + k] = shf[es].T
        recvoh[c, assign_slot[rcv[es]], base + np.arange(k)] = 1.0
        sendrow[c, base:base + k] = tab_row[snd[es]]

    # device edge-slot layout: slot -> (blk, p) with slot = blk*128 + p
    def to_pb(a):   # [NC, ..., ES] -> [NC, 128, ..., NBLK]
        a2 = a.reshape(a.shape[:-1] + (NBLK, EPB))               # [..., NBLK, 128]
        return np.moveaxis(a2, -1, 1)                            # [NC, 128, ..., NBLK]

    geo_in = np.ascontiguousarray(to_pb(geo).reshape(NC, P, 9 * NBLK))   # [NC, 128, (comp,blk)]
    # recv one-hot in (blk, n) layout: [NC, 128, NBLK, SUBN]
    recv_in = np.ascontiguousarray(
        to_pb(recvoh).transpose(0, 1, 3, 2).reshape(NC, P, NBLK * SUBN))
    # gather idx: per subtile 192 slots; idx k at partition k%16, col sub*12 + k//16
    gidx = np.zeros((NC, P, NSUB * 12), dtype=np.int16)
    for c in range(NC):
        w = sendrow[c].reshape(NSUB, 12, 16).astype(np.int16)    # [sub, k//16, k%16]
        packed = w.transpose(2, 0, 1).reshape(16, NSUB * 12)     # [k%16, (sub, k//16)]
        for g in range(8):
            gidx[c, g * 16:(g + 1) * 16, :] = packed
    # per-edge-slot sender species (pad -> 0) in device layout [NC, 128, NBLK]
    sendsp = np.zeros((NC, ES), dtype=np.int64)
    for t in range(TS):
        c = t // NSUB; s = t % NSUB
        es = e_order[bounds[t]:bounds[t + 1]]
        sendsp[c, s * CAP:s * CAP + len(es)] = species[snd[es]]
    sendsp_in = to_pb(sendsp)                                    # [NC, 128, NBLK]
    # per-node-row species (empty rows -> 0; all their uses are masked/zero)
    rowsp = np.zeros((NC, NROW), dtype=np.int64)
    msk = node_of_row >= 0
    rowsp.reshape(-1)[msk] = species[node_of_row[msk]]
    return dict(geo=geo_in, recv=recv_in, gidx=gidx, sendsp=sendsp_in, rowsp=rowsp,
                node_of_row=node_of_row, node_row=node_row)


def _consts():
    blkdiag = ((np.arange(P)[:, None] % 16) == (np.arange(P)[None, :] % 16)).astype(np.float32)
    nrow = np.tile((np.arange(1, N_RBF + 1) * pi / CUTOFF).astype(np.float32)[None, :], (P, 1))
    parc = np.zeros((P, 16), dtype=np.float32)                   # [par, r] keep r where r%2==par
    for par in range(2):
        for r in range(8):
            if r % 2 == par:
                parc[:, par * 8 + r] = 1.0
    sprow = np.tile(np.sqrt(PREF).astype(np.float32)[None, :], (P, 1))   # [128, 20]
    return np.concatenate([blkdiag, nrow, parc, sprow], axis=1)  # [128, 172]


# ================= device program =================
def _build(sim_mode=False):
    PH = int(os.environ.get("KPHASES", "63"))  # bit0 base,1 s1,2 node1,3 repack,4 s2,5 node2
    nc = bacc.Bacc("TRN2", target_bir_lowering=False, debug=False,
                   num_devices=(1 if sim_mode else NC))
    AF = mybir.ActivationFunctionType
    OP = mybir.AluOpType

    # x_main packs [geo 432 | recv 768 | cons 172 | wpack 308 | embse 144]
    NMAIN = 9 * NBLK + NBLK * SUBN + 172 + 308 + NBLK * NAB
    x_main = nc.dram_tensor("x_main", [P, NMAIN], F32, kind="ExternalInput")
    x_gidx = nc.dram_tensor("x_gidx", [P, NSUB * 12], I16, kind="ExternalInput")
    o_b0 = nc.dram_tensor("o_b0", [P, NSUB * 45], F32, kind="ExternalOutput")
    o_b1 = nc.dram_tensor("o_b1", [P, NSUB * 45], F32, kind="ExternalOutput")

    with tile.TileContext(nc) as tc:
        with (
            tc.tile_pool(name="persist", bufs=1) as pp,
            tc.tile_pool(name="work", bufs=2) as wp,
            tc.tile_pool(name="dram", bufs=1, space="DRAM") as dr,
        ):
            # ---------- loads (geo+recv first so the edge phase starts early) ----------
            main = pp.tile([P, NMAIN], F32)
            C1 = 9 * NBLK + NBLK * SUBN
            nc.sync.dma_start(main[:, 0:C1], x_main[:, 0:C1])
            nc.sync.dma_start(main[:, C1:NMAIN], x_main[:, C1:NMAIN])
            gidx = pp.tile([P, NSUB * 12], I16)
            nc.sync.dma_start(gidx[:], x_gidx[:])
            o = 0
            geo = main[:, o:o + 9 * NBLK]; o += 9 * NBLK
            recvs = main[:, o:o + NBLK * SUBN]; o += NBLK * SUBN
            cons = main[:, o:o + 172]; o += 172
            wpack = main[:, o:o + 308]; o += 308
            embsE = main[:, o:o + NBLK * NAB]; o += NBLK * NAB
            blkdiag = cons[:, 0:128]
            nrow = cons[:, 128:136]
            parc = cons[:, 136:152]
            sprow = cons[:, 152:172]
            EM = wpack[:, 212:308]

            # ---------- one-time derived weights ----------
            rtl16 = []
            for l in range(MAX_L + 1):
                rtl_t = pp.tile([P, P], BF16, tag=f"rtl{l}")
                rtl16.append(rtl_t)
                nc.vector.tensor_tensor(
                    out=rtl_t[:].rearrange("p (s n) -> p s n", s=8),
                    in0=wpack[:, l * 8:(l + 1) * 8][:, :, None].to_broadcast([P, 8, 16]),
                    in1=blkdiag.rearrange("p (s n) -> p s n", s=8),
                    op=OP.mult)
            WT16 = pp.tile([P, 180], BF16)
            nc.scalar.copy(WT16[:], wpack[:, 32:212])

            # ---------- per-edge base phase ----------
            D = pp.tile([P, 3 * NBLK], F32)
            nc.vector.tensor_tensor(out=D[:], in0=geo[:, 3 * NBLK:6 * NBLK],
                                    in1=geo[:, 0:3 * NBLK], op=OP.subtract)
            nc.vector.tensor_tensor(out=D[:], in0=D[:], in1=geo[:, 6 * NBLK:9 * NBLK], op=OP.add)
            sq = wp.tile([P, 3 * NBLK], F32, tag="sq")
            nc.vector.tensor_tensor(out=sq[:], in0=D[:], in1=D[:], op=OP.mult)
            r2 = wp.tile([P, NBLK], F32, tag="r2")
            nc.vector.tensor_tensor(out=r2[:], in0=sq[:, 0:NBLK], in1=sq[:, NBLK:2 * NBLK], op=OP.add)
            nc.vector.tensor_tensor(out=r2[:], in0=r2[:], in1=sq[:, 2 * NBLK:3 * NBLK], op=OP.add)
            rr = wp.tile([P, NBLK], F32, tag="rr")
            nc.scalar.activation(rr[:], r2[:], AF.Sqrt)
            rinv = pp.tile([P, NBLK], F32)
            nc.vector.reciprocal(rinv[:], rr[:])
            uu = wp.tile([P, NBLK], F32, tag="uu")
            nc.vector.tensor_scalar_mul(uu[:], rr[:], 1.0 / CUTOFF)
            U = pp.tile([P, 3 * NBLK], F32)
            nc.vector.tensor_tensor(
                out=U[:].rearrange("p (c b) -> p c b", c=3),
                in0=D[:].rearrange("p (c b) -> p c b", c=3),
                in1=rinv[:, None, :].to_broadcast([P, 3, NBLK]), op=OP.mult)
            # bessel args [128, (blk, r)] + range reduction to [-pi, pi): the
            # reduction runs on gpsimd, in parallel with the DVE angular chain
            arg = wp.tile([P, NBLK * 8], F32, tag="arg")
            nc.vector.tensor_tensor(
                out=arg[:].rearrange("p (b r) -> p b r", r=8),
                in0=rr[:, :, None].to_broadcast([P, NBLK, 8]),
                in1=nrow[:, None, :].to_broadcast([P, NBLK, 8]), op=OP.mult)
            ge = wp.tile([P, NBLK * 8], F32, tag="ge")
            for thr, sub in ((4 * pi, 4 * pi), (2 * pi, 2 * pi), (pi, 2 * pi)):
                nc.gpsimd.tensor_scalar(out=ge[:], in0=arg[:], scalar1=float(thr),
                                        scalar2=float(sub), op0=OP.is_ge, op1=OP.mult)
                nc.gpsimd.tensor_tensor(out=arg[:], in0=arg[:], in1=ge[:], op=OP.subtract)
            # angular monomials ang [128, (blk, i)] scaled by sqrt(PREF), and G1,
            # on DVE while gpsimd reduces the bessel arguments
            ang = pp.tile([P, NBLK * N_L], F32)
            angv = ang[:].rearrange("p (b i) -> p b i", i=N_L)
            nc.vector.tensor_scalar(out=angv[:, :, 0], in0=uu[:], scalar1=0.0, scalar2=1.0,
                                    op0=OP.mult, op1=OP.add)
            nc.vector.tensor_copy(
                angv[:, :, 1:4],
                U[:].rearrange("p (c b) -> p b c", c=3))
            for lo, hi, plo, c in _CHAIN_BATCH:
                cnt = hi - lo
                nc.vector.tensor_tensor(
                    out=angv[:, :, lo:hi],
                    in0=angv[:, :, plo:plo + cnt],
                    in1=U[:, c * NBLK:(c + 1) * NBLK][:, :, None].to_broadcast([P, NBLK, cnt]),
                    op=OP.mult)
            nc.vector.tensor_tensor(
                out=angv[:],
                in0=angv[:],
                in1=sprow[:, None, :].to_broadcast([P, NBLK, N_L]),
                op=OP.mult)
            G1 = pp.tile([P, NBLK * N_L * NAB], BF16)

            def build_g1():
                # G1 [128, (blk, i, a)] bf16
                nc.vector.tensor_tensor(
                    out=G1[:].rearrange("p (b i a) -> p b i a", i=N_L, a=NAB),
                    in0=angv[:, :, :, None].to_broadcast([P, NBLK, N_L, NAB]),
                    in1=embsE[:].rearrange("p (b a) -> p b a", a=NAB)[:, :, None, :].to_broadcast([P, NBLK, N_L, NAB]),
                    op=OP.mult)
            build_g1()
            sinv = wp.tile([P, NBLK * 8], F32, tag="sinv")
            nc.scalar.activation(sinv[:], arg[:], AF.Sin)
            # cutoff polynomial
            u2 = wp.tile([P, NBLK], F32, tag="u2")
            nc.vector.tensor_tensor(out=u2[:], in0=uu[:], in1=uu[:], op=OP.mult)
            a1 = wp.tile([P, NBLK], F32, tag="a1")
            nc.vector.tensor_scalar(out=a1[:], in0=uu[:], scalar1=-48.0, scalar2=28.0,
                                    op0=OP.mult, op1=OP.add)
            g21 = wp.tile([P, NBLK], F32, tag="g21")
            nc.vector.tensor_scalar_mul(g21[:], u2[:], 21.0)
            nc.vector.tensor_tensor(out=g21[:], in0=g21[:], in1=a1[:], op=OP.add)
            u6 = wp.tile([P, NBLK], F32, tag="u6")
            nc.vector.tensor_tensor(out=u6[:], in0=u2[:], in1=u2[:], op=OP.mult)
            nc.vector.tensor_tensor(out=u6[:], in0=u6[:], in1=u2[:], op=OP.mult)
            fc = wp.tile([P, NBLK], F32, tag="fc")
            nc.vector.tensor_tensor(out=fc[:], in0=u6[:], in1=g21[:], op=OP.mult)
            nc.vector.tensor_scalar(out=fc[:], in0=fc[:], scalar1=-1.0, scalar2=1.0,
                                    op0=OP.mult, op1=OP.add)
            lt = wp.tile([P, NBLK], F32, tag="lt")
            nc.vector.tensor_scalar(out=lt[:], in0=uu[:], scalar1=1.0, scalar2=None, op0=OP.is_lt)
            nc.vector.tensor_tensor(out=fc[:], in0=fc[:], in1=lt[:], op=OP.mult)
            scal = wp.tile([P, NBLK], F32, tag="scal")
            nc.vector.tensor_tensor(out=scal[:], in0=rinv[:], in1=fc[:], op=OP.mult)
            nc.vector.tensor_scalar_mul(scal[:], scal[:], float(np.sqrt(2.0 / CUTOFF)))
            rc = pp.tile([P, NBLK * 8], F32)
            nc.vector.tensor_tensor(
                out=rc[:].rearrange("p (b r) -> p b r", r=8),
                in0=sinv[:].rearrange("p (b r) -> p b r", r=8),
                in1=scal[:, :, None].to_broadcast([P, NBLK, 8]), op=OP.mult)
            # parity-masked rc: rcEO [128, (blk, par, r)]
            rcEO = pp.tile([P, NBLK * 16], F32)
            nc.vector.tensor_tensor(
                out=rcEO[:].rearrange("p (b q r) -> p b q r", q=2, r=8),
                in0=rc[:].rearrange("p (b r) -> p b r", r=8)[:, :, None, :].to_broadcast([P, NBLK, 2, 8]),
                in1=parc.rearrange("p (q r) -> p q r", q=2)[:, None, :, :].to_broadcast([P, NBLK, 2, 8]),
                op=OP.mult)
            # unified parity-split S_w, bf16, persistent (used by both stages);
            # built per 6-block group inside the stage-1 loop, alternating
            # DVE/gpsimd to balance engine load
            sw2 = pp.tile([P, NBLK * 256], BF16)

            def build_sw2(g4):
                bs = slice(g4 * 6, g4 * 6 + 6)
                eng = nc.vector
                eng.tensor_tensor(
                    out=sw2[:, g4 * 1536:(g4 + 1) * 1536].rearrange(
                        "p (b qr n) -> p b qr n", qr=16, n=16),
                    in0=recvs[:].rearrange("p (b n) -> p b n", n=SUBN)[:, bs, None, :].to_broadcast([P, 6, 16, 16]),
                    in1=rcEO[:].rearrange("p (b qr) -> p b qr", qr=16)[:, bs, :, None].to_broadcast([P, 6, 16, 16]),
                    op=OP.mult)

            def symmetrize_pool(bv, av, sv, ns):
                # bv [P,ns,5,c]; av/sv [P,ns,20,c]: sum-of-squares tree on gpsimd
                s5 = wp.tile([P, 8 * 5 * CHAN], F32, tag="ps5")
                v5 = s5[:].rearrange("p (s i c) -> p s i c", i=5, c=CHAN)[:, 0:ns]
                s3 = wp.tile([P, 8 * 3 * CHAN], F32, tag="ps3")
                v3 = s3[:].rearrange("p (s i c) -> p s i c", i=3, c=CHAN)[:, 0:ns]
                TT = nc.gpsimd.tensor_tensor
                nc.vector.tensor_copy(bv[:, :, 0, :], av[:, :, 0, :])
                nc.gpsimd.tensor_copy(bv[:, :, 1, :], sv[:, :, 0, :])
                # l=1: i 1..4
                TT(out=v3[:, :, 0, :], in0=sv[:, :, 1, :], in1=sv[:, :, 2, :], op=OP.add)
                TT(out=bv[:, :, 2, :], in0=v3[:, :, 0, :], in1=sv[:, :, 3, :], op=OP.add)
                # l=2: i 4..10
                TT(out=v3[:], in0=sv[:, :, 4:7, :], in1=sv[:, :, 7:10, :], op=OP.add)
                TT(out=v5[:, :, 0, :], in0=v3[:, :, 0, :], in1=v3[:, :, 1, :], op=OP.add)
                TT(out=bv[:, :, 3, :], in0=v5[:, :, 0, :], in1=v3[:, :, 2, :], op=OP.add)
                # l=3: i 10..20
                TT(out=v5[:], in0=sv[:, :, 10:15, :], in1=sv[:, :, 15:20, :], op=OP.add)
                TT(out=v3[:, :, 0:2, :], in0=v5[:, :, 0:2, :], in1=v5[:, :, 2:4, :], op=OP.add)
                TT(out=v3[:, :, 2, :], in0=v3[:, :, 0, :], in1=v3[:, :, 1, :], op=OP.add)
                TT(out=bv[:, :, 4, :], in0=v3[:, :, 2, :], in1=v5[:, :, 4, :], op=OP.add)

            def symmetrize_dve(bv, av, sv):
                nc.vector.tensor_copy(bv[:, :, 0, :], av[:, :, 0, :])
                for l, (a, b) in enumerate(L_RANGES):
                    nc.vector.tensor_reduce(
                        out=bv[:, :, l + 1, :],
                        in_=sv[:, :, a:b, :].transpose([0, 1, 3, 2]),
                        axis=mybir.AxisListType.X, op=OP.add)

            A_all = pp.tile([P, NSUB * 180], F32)
            A16 = pp.tile([P, NSUB * 180], BF16)
            mem16 = pp.tile([P, NSUB * 180], BF16)
            B0_all = pp.tile([P, NSUB * 45], F32)
            B1_all = pp.tile([P, NSUB * 45], F32)
            red1 = pp.tile([P, NSUB * CHAN], F32)
            chic = pp.tile([16, NSUB * CHAN], F32)
            Vsb = pp.tile([16, NSUB * CHAN], BF16)

            tabsh = dr.tile([NROW, TABW], BF16)
            tabfull = dr.tile([NC * NROW, TABW], BF16)

            # ---------- stage 1: seg-sum + RT + A; node-level per 8 subtiles ----------
            s1ctx = tc.tile_pool(name="ps_s1", bufs=2, space="PSUM")
            ps_s1 = s1ctx.__enter__()
            if PH & 2:
                build_sw2(0)
                build_sw2(1)
            for g8 in range(4 if (PH & 2) else 0):
                if g8 < 3:
                    # prefetch next group's S_w on DVE before this group's
                    # node-level ops (which wait on PE/Act) enter the queue
                    build_sw2(2 * g8 + 2)
                    build_sw2(2 * g8 + 3)
                t0g = ps_s1.tile([P, 480], F32, space="PSUM", tag="t0g")
                for j in range(8):
                    s = g8 * 8 + j
                    osl = slice(j * 60, (j + 1) * 60)
                    mms = [(blk, p0, p1, q) for (blk, p0, p1) in _block_ranges(s) for q in (0, 1)]
                    for mi, (blk, p0, p1, q) in enumerate(mms):
                        nc.tensor.matmul(t0g[:, osl], lhsT=sw2[p0:p1, blk * 256 + q * 128: blk * 256 + q * 128 + 128],
                                         rhs=G1[p0:p1, blk * 60:(blk + 1) * 60],
                                         start=(mi == 0), stop=(mi == len(mms) - 1))
                t0c = wp.tile([P, 480], BF16, tag="t0c")
                nc.scalar.copy(t0c[:], t0g[:])
                t1g = ps_s1.tile([P, 480], F32, space="PSUM", tag="t1g")
                t0cv = t0c[:].rearrange("p (j f) -> p j f", f=60)
                t1gv = t1g[:].rearrange("p (j f) -> p j f", f=60)
                for l, (a, b) in enumerate(L_RANGES):
                    nc.tensor.matmul(t1gv[:, :, a * NAB:b * NAB],
                                     lhsT=rtl16[l][:], rhs=t0cv[:, :, a * NAB:b * NAB],
                                     start=True, stop=True)
                if not (PH & 4):
                    continue
                # ---- group node-level: A, A16, B0, chi, V, repack, AG slice ----
                sl = slice(g8 * 1440, (g8 + 1) * 1440)
                sl45 = slice(g8 * 360, (g8 + 1) * 360)
                sl9 = slice(g8 * 72, (g8 + 1) * 72)
                nc.vector.tensor_tensor(
                    out=A_all[:, sl].rearrange("p (j ia b) -> p j ia b", j=8, b=NAB),
                    in0=t1g[:].rearrange("p (j ia) -> p j ia", j=8)[:, :, :, None].to_broadcast([P, 8, 60, NAB]),
                    in1=EM[:, g8 * 24:(g8 + 1) * 24].rearrange("p (j b) -> p j b", b=NAB)[:, :, None, :].to_broadcast([P, 8, 60, NAB]),
                    op=OP.mult)
                nc.scalar.mul(A16[:, sl], A_all[:, sl], float(MP_NORM))
                scr = wp.tile([P, 1440], F32, tag="scr")
                nc.scalar.activation(scr[:], A_all[:, sl], AF.Square)
                bv = B0_all[:, sl45].rearrange("p (s l c) -> p s l c", l=5, c=CHAN)
                av = A_all[:, sl].rearrange("p (s i c) -> p s i c", i=N_L, c=CHAN)
                sv = scr[:].rearrange("p (s i c) -> p s i c", i=N_L, c=CHAN)
                if g8 == 3:
                    # last group: chi computed straight from the squares so the
                    # final AG slice doesn't wait on the symmetrize tree
                    rv = red1[:, sl9].rearrange("p (s c) -> p s c", c=CHAN)
                    nc.vector.tensor_reduce(
                        out=rv, in_=sv.transpose([0, 1, 3, 2]),
                        axis=mybir.AxisListType.X, op=OP.add)
                    nc.vector.tensor_tensor(out=rv, in0=rv, in1=av[:, :, 0, :], op=OP.add)
                    symmetrize_pool(bv, av, sv, 8)
                else:
                    symmetrize_pool(bv, av, sv, 8)
                    nc.vector.tensor_reduce(
                        out=red1[:, sl9].rearrange("p (s c) -> p s c", c=CHAN),
                        in_=bv.transpose([0, 1, 3, 2]),
                        axis=mybir.AxisListType.X, op=OP.add)
                chips = ps_s1.tile([16, 72], F32, space="PSUM", tag="chips")
                nc.tensor.matmul(chips[:], lhsT=blkdiag[:, 0:16], rhs=red1[:, sl9],
                                 start=True, stop=True)
                nc.vector.tensor_scalar_mul(chic[:, sl9], chips[:], float(MP_NORM))
                nc.vector.tensor_tensor(
                    out=Vsb[:, sl9].rearrange("p (s a b) -> p s a b", a=NAB, b=NAB),
                    in0=chic[:, sl9].rearrange("p (s a b) -> p s a b", a=NAB, b=NAB),
                    in1=EM[0:16, g8 * 24:(g8 + 1) * 24].rearrange("p (s a) -> p s a", a=NAB)[:, :, :, None].to_broadcast([16, 8, NAB, NAB]),
                    op=OP.mult)
                if (PH & 8) and g8 % 2 == 1:
                    # repack super-group: A rows + V column for 16 subtiles
                    # (256 table rows), 8+1 DMAs; then one AllGather slice
                    sg = g8 // 2
                    ssl = slice(sg * 2880, (sg + 1) * 2880)
                    ssl9 = slice(sg * 144, (sg + 1) * 144)
                    for sp in range(8):
                        nc.sync.dma_start(
                            out=tabsh[:].rearrange("(s n) w -> n s w", n=SUBN)[:, sg * 16:(sg + 1) * 16, sp * 180:(sp + 1) * 180],
                            in_=A16[sp * 16:(sp + 1) * 16, ssl].rearrange("n (s f) -> n s f", f=180))
                    nc.sync.dma_start(
                        out=tabsh[:].rearrange("(s n) w -> n s w", n=SUBN)[:, sg * 16:(sg + 1) * 16, 1440:1449],
                        in_=Vsb[:, ssl9].rearrange("n (s c) -> n s c", c=CHAN))
                    rsl = slice(sg * 256, (sg + 1) * 256)
                    if sim_mode:
                        # stand-in for the sliced AllGather: one DMA per slice
                        # re-reading the shard 4x models the measured ~17us
                        # 8-core AG of the 1.5MB/rank shard (same total bytes)
                        nc.sync.dma_start(
                            tabfull[sg * NC * 256: sg * NC * 256 + 1024, :].rearrange(
                                "(c r) w -> c r w", c=4),
                            tabsh[rsl, :][None, :, :].to_broadcast([4, 256, TABW]))
                    else:
                        # tabfull rows are (slice, core, 256): slice output is
                        # the contiguous rank-major block for this slice
                        nc.gpsimd.collective_compute(
                            "AllGather", mybir.AluOpType.bypass,
                            replica_groups=[list(range(NC))],
                            ins=[tabsh[rsl, :]],
                            outs=[tabfull[sg * NC * 256:(sg + 1) * NC * 256, :]])
            nc.sync.dma_start(o_b0[:], B0_all[:])
            # memory term (bf16 fast path; WT pre-divided by MP_NORM on host)
            nc.vector.tensor_tensor(
                out=mem16[:].rearrange("p (s f) -> p s f", f=180),
                in0=A16[:].rearrange("p (s f) -> p s f", f=180),
                in1=WT16[:, None, :].to_broadcast([P, NSUB, 180]),
                op=OP.mult)
            s1ctx.__exit__(None, None, None)

            # ---------- stage 2 ----------
            s2ctx = tc.tile_pool(name="ps_s2", bufs=2, space="PSUM")
            ps_s2 = s2ctx.__enter__()
            A1f = None
            for gg in range(NSUB // GB if (PH & 16) else 0):
                g8 = gg // 4
                if gg % 2 == 0:
                    A1f = wp.tile([P, 720], F32, tag="a1f", bufs=3)
                gat = wp.tile([P, 3, TABW], BF16, tag="gat", bufs=4)
                nc.gpsimd.dma_gather(gat[:], tabfull[:],
                                     gidx[:, gg * 24:(gg + 1) * 24],
                                     GB * CAP, GB * CAP, TABW)
                G2 = wp.tile([P, 3, 180], BF16, tag="g2", bufs=3)
                nc.vector.tensor_tensor(
                    out=G2[:].rearrange("p b (i c) -> p b i c", c=CHAN),
                    in0=angv[:, 3 * gg:3 * gg + 3, :][:, :, :, None].to_broadcast([P, 3, N_L, CHAN]),
                    in1=gat[:, :, 1440:1449][:, :, None, :].to_broadcast([P, 3, N_L, CHAN]),
                    op=OP.mult)
                t2pair = ps_s2.tile([P, 360], F32, space="PSUM", tag="t2", bufs=3)
                a1pair = ps_s2.tile([P, 360], F32, space="PSUM", tag="a1p", bufs=3)
                t2s = wp.tile([P, 360], BF16, tag="t2s", bufs=3)
                for s2 in range(GB):
                    s = gg * GB + s2
                    osl = slice(s2 * 180, (s2 + 1) * 180)
                    ranges = _block_ranges(s)
                    mms = [(blk, p0, p1, q) for (blk, p0, p1) in ranges for q in (0, 1)]
                    for mi, (blk, p0, p1, q) in enumerate(mms):
                        nc.tensor.matmul(
                            t2pair[:, osl],
                            lhsT=sw2[p0:p1, blk * 256 + q * 128: blk * 256 + q * 128 + 128],
                            rhs=G2[p0:p1, blk - 3 * gg, :],
                            start=(mi == 0), stop=(mi == len(mms) - 1))
                    for ri, (blk, p0, p1) in enumerate(ranges):
                        bloc = blk - 3 * gg
                        for sig in (0, 2, 4, 6, 1, 3, 5, 7):
                            k, par = sig // 2, sig % 2
                            nc.tensor.matmul(
                                a1pair[k * 32:(k + 1) * 32, osl],
                                lhsT=sw2[p0:p1, blk * 256 + par * 128 + k * 32: blk * 256 + par * 128 + (k + 1) * 32],
                                rhs=gat[p0:p1, bloc, sig * 180:(sig + 1) * 180],
                                start=(ri == 0 and par == 0), stop=False,
                                tile_position=(p0, k * 32))
                    nc.vector.tensor_tensor(
                        out=t2s[:, osl].rearrange("p (i a b) -> p i a b", a=NAB, b=NAB),
                        in0=t2pair[:, osl].rearrange("p (i a b) -> p i a b", a=NAB, b=NAB),
                        in1=EM[:, s * NAB:(s + 1) * NAB][:, None, None, :].to_broadcast([P, N_L, NAB, NAB]),
                        op=OP.mult)
                t2sv = t2s[:].rearrange("p (s2 f) -> p s2 f", f=180)
                a1v = a1pair[:].rearrange("p (s2 f) -> p s2 f", f=180)
                for l, (a, b) in enumerate(L_RANGES):
                    nc.tensor.matmul(a1v[:, :, a * CHAN:b * CHAN],
                                     lhsT=rtl16[l][:], rhs=t2sv[:, :, a * CHAN:b * CHAN],
                                     start=False, stop=(l == MAX_L))
                nc.vector.tensor_tensor(
                    out=A1f[:, (gg % 2) * 360:(gg % 2 + 1) * 360],
                    in0=a1pair[:], in1=mem16[:, gg * 360:(gg + 1) * 360], op=OP.add)
                if gg % 2 < 1 or not (PH & 32):
                    continue
                # ---- stage 2 node-level per 4 subtiles: B1 + output ----
                h4 = gg // 2
                sl45 = slice(h4 * 180, (h4 + 1) * 180)
                scr = wp.tile([P, 720], F32, tag="scr1", bufs=3)
                nc.scalar.activation(scr[:], A1f[:], AF.Square)
                bv = B1_all[:, sl45].rearrange("p (s l c) -> p s l c", l=5, c=CHAN)
                av = A1f[:].rearrange("p (s i c) -> p s i c", i=N_L, c=CHAN)
                sv = scr[:].rearrange("p (s i c) -> p s i c", i=N_L, c=CHAN)
                symmetrize_dve(bv, av, sv)
                nc.sync.dma_start(o_b1[:, sl45], B1_all[:, sl45])
            s2ctx.__exit__(None, None, None)
            if not (PH & 4):
                nc.sync.dma_start(o_b0[:, 0:172], cons[:])
            if not (PH & 32):
                nc.sync.dma_start(o_b1[:, 0:172], cons[:])
    nc.compile()
    return nc


# ================= public entry =================
def kernel(positions, shifts, W_emb, W_rt, W_nm, atomic_numbers, edge_index):
    global _PROGRAM
    prep = _prep(positions, shifts, atomic_numbers, edge_index)
    consts = _consts()
    if _PROGRAM is None:
        _PROGRAM = _build()
    nc = _PROGRAM
    wemb = np.asarray(W_emb, dtype=np.float32)
    wrt = np.asarray(W_rt, dtype=np.float32)
    wnm = np.asarray(W_nm, dtype=np.float32)
    # host-replicated weight patterns (pure tiling/gathers of the small weights)
    pg = np.arange(P) // 16                                   # r|s' group per partition
    rtlw = wrt[:, pg, :].transpose(1, 0, 2).reshape(P, 32)    # [p, (l, s')] = W_rt[l, p//16, s']
    wtp = wnm[0, pg][:, L_OF, :].reshape(P, 180) / np.float32(MP_NORM)
    in_maps = []
    for c in range(NC):
        em = wemb[prep["rowsp"][c].reshape(NSUB, SUBN)]       # [sub, n, a]
        em = em[:, np.arange(P) % 16, :].transpose(1, 0, 2).reshape(P, NSUB * NAB)
        wpack = np.concatenate([rtlw, wtp, em], axis=1).astype(np.float32)
        embse = wemb[prep["sendsp"][c]].reshape(P, NBLK * NAB).astype(np.float32)
        main = np.ascontiguousarray(np.concatenate(
            [prep["geo"][c], prep["recv"][c], consts, wpack, embse],
            axis=1).astype(np.float32))
        in_maps.append(dict(x_main=main, x_gidx=prep["gidx"][c]))
    res = run_bass_kernel_spmd(nc, in_maps, list(range(NC))).results
    # unshard: [128=(s',n), (sub, l, c)] -> node rows
    out = np.zeros((N_NODES, N_RB, 5, CHAN, 2), dtype=np.float32)
    node_of_row = prep["node_of_row"]
    for c in range(NC):
        for mp, name in ((0, "o_b0"), (1, "o_b1")):
            arr = res[c][name].reshape(8, SUBN, NSUB, 5, CHAN)    # [s', n, sub, l, ch]
            rows = arr.transpose(2, 1, 0, 3, 4).reshape(NROW, N_RB, 5, CHAN)
            valid = node_of_row[c * NROW:(c + 1) * NROW] >= 0
            out[node_of_row[c * NROW:(c + 1) * NROW][valid], :, :, :, mp] = rows[valid]
    return out
